# revision 34
# baseline (speedup 1.0000x reference)
"""Trainium2 Bass kernel for DiscreteRotation (moe_routing).

Per sample: k = argmax(mean_hw(x) @ W + b); out = rot90(x, k, axes=(H,W)).

Pure data parallel over 8 NeuronCores (8 samples each). Runtime branching is
not supported by this execution path, so the routing is SPECULATED: the
classifier bias dominates the near-zero image means of this input regime, so
k_prior = argmax(b) predicts every sample's route. A single static HW launch
rotates every sample per the speculated pattern AND computes the true logits
from the same SBUF-resident data (per-channel pixel sums on DVE, ones-matmul
on PE); the host then argmaxes the logits and fixes up any mispredicted
sample with numpy rot90 (correct for arbitrary inputs, never triggered by
the bias-dominated regime).

Per-slot rotation code (all fully static):
  r=0: load -> store.
  r=1: load -> pixel-reverse within rows (DVE/ACT, negative-stride input
       APs) -> pixel-transpose on PE (per-channel [<=128 x <=128] tiles via
       identity matmul, PSUM -> DVE/ACT copy) -> store.
  r=2: rot90 applied twice.
  r=3: pixel-transpose, copy-out at reversed pixel positions, each output
       half stored as soon as its copies land -> store.
A separate two-launch path (classify program + pattern-compiled rotate
program) is kept for reference/testing.

Every DMA descriptor is a contiguous 28KB image row on both HBM and SBUF
sides, so all transfers run at full HBM rate. SBUF row layout: image row r
lives at partition r%128, slot r//128 (slot0: rows 0..127, slot1: rows
128..223).
"""
import numpy as np
from contextlib import ExitStack

import concourse.bass as bass
import concourse.bacc as bacc
import concourse.tile as tile
import concourse.mybir as mybir
from concourse.bass_utils import run_bass_kernel_spmd

F32 = mybir.dt.float32

N_CORES = 8
H = 224
W = 224
C = 32
ROW = W * C          # 7168 f32 per image row (28672 B)
P0, P1 = 128, 96     # rows in slot 0 / slot 1


def _flip(ap: bass.AP, dim: int) -> bass.AP:
    """Reverse iteration order of one AP dim."""
    pairs = [list(p) for p in ap.ap]
    stride, num = pairs[dim]
    off = ap.offset + stride * (num - 1)
    pairs[dim] = [-stride, num]
    return bass.AP(ap.tensor, off, pairs)


def _bcast_mid(ap2d: bass.AP, n: int) -> bass.AP:
    """[p, f] -> [p, n, f] with stride-0 middle dim."""
    pairs = [list(p) for p in ap2d.ap]
    assert len(pairs) == 2
    return bass.AP(ap2d.tensor, ap2d.offset, [pairs[0], [0, n], pairs[1]])


def _pixview(ap2d: bass.AP) -> bass.AP:
    """[p, ROW-range] -> [p, c(32), j(224)] channel-major pixel view."""
    return ap2d.rearrange("p (j c) -> p c j", j=W, c=C)


# load chunks: (row0, npart, part0, slot)
CHUNKS = [(0, 64, 0, 0), (64, 64, 64, 0), (128, 64, 0, 1), (192, 32, 64, 1)]


def _build_classify(S: int) -> bacc.Bacc:
    nc = bacc.Bacc("TRN2", target_bir_lowering=False, debug=False,
                   num_devices=N_CORES)
    x = nc.dram_tensor("x", [S * H, ROW], F32, kind="ExternalInput").ap()
    wts = nc.dram_tensor("wts", [1, 4 * C], F32, kind="ExternalInput").ap()
    bias = nc.dram_tensor("bias", [1, 4], F32, kind="ExternalInput").ap()
    lg = nc.dram_tensor("lg", [S, 4], F32, kind="ExternalOutput").ap()

    with tile.TileContext(nc) as tc:
        with ExitStack() as ctx:
            cpool = ctx.enter_context(tc.tile_pool(name="consts", bufs=1))
            apool = ctx.enter_context(tc.tile_pool(name="A", bufs=3))
            spool = ctx.enter_context(tc.tile_pool(name="small", bufs=2))
            mpool = ctx.enter_context(
                tc.tile_pool(name="pmean", bufs=2, space="PSUM"))

            wt = cpool.tile([1, 4 * C], F32)
            bt = cpool.tile([1, 4], F32)
            ones = cpool.tile([128, 1], F32)
            nc.sync.dma_start(out=wt[:], in_=wts)
            nc.sync.dma_start(out=bt[:], in_=bias)
            nc.vector.memset(ones[:], 1.0)

            for s in range(S):
                A = apool.tile([128, 2 * ROW], F32, name=f"A{s}", tag="A")
                sums = spool.tile([128, 2 * C], F32, name=f"sm{s}", tag="sm")
                for r0, n, p0, sl in CHUNKS:
                    nc.sync.dma_start(
                        out=A[p0:p0 + n, sl * ROW:sl * ROW + ROW],
                        in_=x[s * H + r0:s * H + r0 + n, :])
                # per slot: contiguous halving adds (full 128-lane width,
                # DVE 2x mode) down to 14 pixels, then a small strided
                # per-channel reduce
                for sl, n in ((0, P0), (1, P1)):
                    base = sl * ROW
                    npix = W
                    while npix % 2 == 0 and npix > 14:
                        half = npix // 2 * C
                        nc.vector.tensor_add(
                            out=A[0:n, base:base + half],
                            in0=A[0:n, base:base + half],
                            in1=A[0:n, base + half:base + 2 * half])
                        npix //= 2
                    av = A[0:n, base:base + npix * C]
                    nc.vector.tensor_reduce(
                        out=sums[0:n, sl * C:(sl + 1) * C],
                        in_=bass.AP(av.tensor, av.offset,
                                    [list(av.ap[0]), [1, C], [C, npix]]),
                        axis=mybir.AxisListType.X, op=mybir.AluOpType.add)
                pm = mpool.tile([1, C], F32, name=f"pm{s}", tag="pm")
                nc.tensor.matmul(pm[0:1, 0:C], lhsT=ones[0:128, 0:1],
                                 rhs=sums[0:128, 0:C], start=True, stop=False)
                nc.tensor.matmul(pm[0:1, 0:C], lhsT=ones[0:96, 0:1],
                                 rhs=sums[0:96, C:2 * C], start=False,
                                 stop=True)
                tmp = spool.tile([1, 4 * C], F32, name=f"tp{s}", tag="tp")
                wv = wt[:].rearrange("p (k c) -> p k c", k=4, c=C)
                nc.vector.tensor_mul(
                    out=tmp[:].rearrange("p (k c) -> p k c", k=4, c=C),
                    in0=_bcast_mid(pm[0:1, 0:C], 4), in1=wv)
                t4 = spool.tile([1, 4], F32, name=f"t4{s}", tag="t4")
                nc.vector.tensor_reduce(
                    out=t4[:],
                    in_=tmp[:].rearrange("p (k c) -> p k c", k=4, c=C),
                    axis=mybir.AxisListType.X, op=mybir.AluOpType.add)
                l4 = spool.tile([1, 4], F32, name=f"l4{s}", tag="l4")
                nc.vector.tensor_add(out=l4[:], in0=t4[:], in1=bt[:])
                nc.sync.dma_start(out=lg[s:s + 1, :], in_=l4[:])
    nc.finalize()
    return nc


def _build_rotate(S: int, pattern: tuple, with_cls: bool = False) -> bacc.Bacc:
    """Static program: sample slot s gets rotation pattern[s].

    with_cls: additionally compute per-sample logits from the loaded data
    (per-channel pixel sums on DVE, ones-matmul on PE, logit chain) and
    write them to "lg" — lets the host verify a speculated pattern without
    a separate classification pass over x.
    """
    nc = bacc.Bacc("TRN2", target_bir_lowering=False, debug=False,
                   num_devices=N_CORES)
    x = nc.dram_tensor("x", [S * H, ROW], F32, kind="ExternalInput").ap()
    ident = nc.dram_tensor("ident", [128, 128], F32, kind="ExternalInput").ap()
    y = nc.dram_tensor("y", [S * H, ROW], F32, kind="ExternalOutput").ap()
    if with_cls:
        wts = nc.dram_tensor("wts", [1, 4 * C], F32, kind="ExternalInput").ap()
        bias = nc.dram_tensor("bias", [1, 4], F32, kind="ExternalInput").ap()
        lg = nc.dram_tensor("lg", [S, 4], F32, kind="ExternalOutput").ap()

    with tile.TileContext(nc) as tc:
        with ExitStack() as ctx:
            cpool = ctx.enter_context(tc.tile_pool(name="consts", bufs=1))
            apool = ctx.enter_context(tc.tile_pool(name="A", bufs=2))
            bpool = ctx.enter_context(tc.tile_pool(name="B", bufs=1))
            tpool = ctx.enter_context(
                tc.tile_pool(name="ptrans", bufs=6, space="PSUM"))

            it = cpool.tile([128, 128], F32)
            nc.sync.dma_start(out=it[:], in_=ident)
            if with_cls:
                spool = ctx.enter_context(tc.tile_pool(name="small", bufs=2))
                mpool = ctx.enter_context(
                    tc.tile_pool(name="pmean", bufs=2, space="PSUM"))
                wt = cpool.tile([1, 4 * C], F32)
                bt = cpool.tile([1, 4], F32)
                ones = cpool.tile([128, 1], F32)
                nc.sync.dma_start(out=wt[:], in_=wts)
                nc.sync.dma_start(out=bt[:], in_=bias)
                nc.vector.memset(ones[:], 1.0)

            # per-sample load/reduce quarters: (slot, window, nrows, npix)
            QUARTS = [(0, 0, P0, P0), (1, 0, P1, P0),
                      (0, 1, P0, P1), (1, 1, P1, P1)]

            def classify(s, A):
                """Per-channel pixel sums per loaded quarter (full-lane
                strided reduces; each starts as soon as its quarter lands),
                accumulated across quarters by PE -> logits row s."""
                sums = spool.tile([128, 4 * C], F32, name=f"sm{s}", tag="sm")
                for q, (sl, w_, n, npx) in enumerate(QUARTS):
                    off = sl * ROW + w_ * 128 * C
                    av = A[0:n, off:off + npx * C]
                    nc.vector.tensor_reduce(
                        out=sums[0:n, q * C:(q + 1) * C],
                        in_=av.rearrange("p (j c) -> p c j", j=npx, c=C),
                        axis=mybir.AxisListType.X, op=mybir.AluOpType.add)
                pm = mpool.tile([1, C], F32, name=f"pm{s}", tag="pm")
                for q, (sl, w_, n, npx) in enumerate(QUARTS):
                    nc.tensor.matmul(pm[0:1, 0:C], lhsT=ones[0:n, 0:1],
                                     rhs=sums[0:n, q * C:(q + 1) * C],
                                     start=(q == 0), stop=(q == 3))
                tmp = spool.tile([1, 4 * C], F32, name=f"tp{s}", tag="tp")
                wv = wt[:].rearrange("p (k c) -> p k c", k=4, c=C)
                nc.vector.tensor_mul(
                    out=tmp[:].rearrange("p (k c) -> p k c", k=4, c=C),
                    in0=_bcast_mid(pm[0:1, 0:C], 4), in1=wv)
                t4 = spool.tile([1, 4], F32, name=f"t4{s}", tag="t4")
                nc.vector.tensor_reduce(
                    out=t4[:],
                    in_=tmp[:].rearrange("p (k c) -> p k c", k=4, c=C),
                    axis=mybir.AxisListType.X, op=mybir.AluOpType.add)
                l4 = spool.tile([1, 4], F32, name=f"l4{s}", tag="l4")
                nc.vector.tensor_add(out=l4[:], in0=t4[:], in1=bt[:])
                nc.sync.dma_start(out=lg[s:s + 1, :], in_=l4[:])

            def load(s, A):
                # quarter loads in window-major order so the w=0 transposes
                # (and their reduces) start as soon as their data lands
                for sl, w_, n, npx in QUARTS:
                    off = sl * ROW + w_ * 128 * C
                    xr = x[s * H + sl * 128:s * H + sl * 128 + n,
                           w_ * 128 * C:w_ * 128 * C + npx * C]
                    nc.sync.dma_start(out=A[0:n, off:off + npx * C], in_=xr)

            def store_fwd(s, src):
                nc.sync.dma_start(out=y[s * H:s * H + P0, :],
                                  in_=src[0:P0, 0:ROW])
                nc.sync.dma_start(out=y[s * H + P0:s * H + H, :],
                                  in_=src[0:P1, ROW:2 * ROW])

            def rev_pixels(src, dst):
                for sl, n, eng in ((0, P0, "dve"), (1, P1, "act")):
                    sv = src[0:n, sl * ROW:sl * ROW + ROW].rearrange(
                        "p (j c) -> p j c", j=W, c=C)
                    dv = dst[0:n, sl * ROW:sl * ROW + ROW].rearrange(
                        "p (j c) -> p j c", j=W, c=C)
                    if eng == "dve":
                        nc.vector.tensor_copy(out=dv, in_=_flip(sv, 1))
                    else:
                        nc.scalar.copy(out=dv, in_=_flip(sv, 1))

            def transpose_pass(s, src, dst, mode, store=None):
                # w outer so each output half completes (and can store)
                # while the other half is still being transposed
                for w, fw in ((0, P0), (1, P1)):  # dst row window
                    for sl, ps in ((0, P0), (1, P1)):   # source row slot
                        for g in range(8):            # channel groups of 4
                            pt = tpool.tile([128, 512], F32,
                                            name=f"pt{s}{g}{sl}{w}", tag="pt")
                            sv = _pixview(src[0:ps, sl * ROW:sl * ROW + ROW])
                            for cc in range(4):
                                ch = g * 4 + cc
                                nc.tensor.transpose(
                                    pt[0:fw, cc * ps:(cc + 1) * ps],
                                    sv[0:ps, ch:ch + 1, w * 128:w * 128 + fw],
                                    it[0:ps, 0:ps])
                            dv = _pixview(dst[0:fw, w * ROW:w * ROW + ROW])
                            if mode == "T":
                                d3 = dv[0:fw, g * 4:(g + 1) * 4,
                                        sl * 128:sl * 128 + ps]
                            else:  # "k3": reversed pixel positions
                                j0 = 96 if sl == 0 else 0
                                d3 = _flip(
                                    dv[0:fw, g * 4:(g + 1) * 4, j0:j0 + ps], 2)
                            src3 = pt[0:fw, 0:4 * ps].rearrange(
                                "p (c j) -> p c j", c=4, j=ps)
                            if g % 2 == 0:
                                nc.vector.tensor_copy(out=d3, in_=src3)
                            else:
                                nc.scalar.copy(out=d3, in_=src3)
                    if store is not None:
                        store(w)

            A_tiles = {}

            def get_A(s):
                if s not in A_tiles:
                    A_tiles[s] = apool.tile([128, 2 * ROW], F32,
                                            name=f"A{s}", tag="A")
                return A_tiles[s]

            load(0, get_A(0))
            for s in range(S):
                A = get_A(s)
                if with_cls:
                    classify(s, A)
                if s + 1 < S:
                    load(s + 1, get_A(s + 1))
                r = pattern[s]
                if r == 0:
                    store_fwd(s, A)
                    continue
                B = bpool.tile([128, 2 * ROW], F32, name=f"B{s}", tag="B")
                if r == 1:
                    rev_pixels(A, B)
                    transpose_pass(s, B, A, "T")
                    store_fwd(s, A)
                elif r == 2:
                    rev_pixels(A, B)
                    transpose_pass(s, B, A, "T")
                    rev_pixels(A, B)
                    transpose_pass(s, B, A, "T")
                    store_fwd(s, A)
                else:  # r == 3
                    def store_w(w, s=s, B=B):
                        if w == 0:
                            nc.sync.dma_start(out=y[s * H:s * H + P0, :],
                                              in_=B[0:P0, 0:ROW])
                        else:
                            nc.sync.dma_start(
                                out=y[s * H + P0:s * H + H, :],
                                in_=B[0:P1, ROW:2 * ROW])
                    transpose_pass(s, A, B, "k3", store=store_w)
    nc.finalize()
    return nc


_NC_CACHE = {}


def get_classify_nc(S):
    key = ("cls", S)
    if key not in _NC_CACHE:
        _NC_CACHE[key] = _build_classify(S)
    return _NC_CACHE[key]


def get_rotate_nc(S, pattern):
    key = ("rot", S, pattern)
    if key not in _NC_CACHE:
        _NC_CACHE[key] = _build_rotate(S, pattern)
    return _NC_CACHE[key]


def get_rotate_cls_nc(S, pattern):
    key = ("rotcls", S, pattern)
    if key not in _NC_CACHE:
        _NC_CACHE[key] = _build_rotate(S, pattern, with_cls=True)
    return _NC_CACHE[key]


def run_rotate_cls(x, pattern):
    """One launch: rotate per the speculated pattern AND emit logits."""
    B = x.shape[0]
    S = B // N_CORES
    ident = np.eye(128, dtype=np.float32)
    wts = np.ascontiguousarray(
        (_W_CLS / float(H * W)).T.reshape(1, 4 * C)).astype(np.float32)
    bias = np.ascontiguousarray(_B_CLS.reshape(1, 4)).astype(np.float32)
    in_maps = []
    for c in range(N_CORES):
        xs = np.ascontiguousarray(x[c * S:(c + 1) * S].reshape(S * H, ROW))
        in_maps.append({"x": xs, "ident": ident, "wts": wts, "bias": bias})
    nc = get_rotate_cls_nc(S, pattern)
    res = run_bass_kernel_spmd(nc, in_maps, core_ids=list(range(N_CORES)))
    out = np.empty((B, H, W, C), dtype=np.float32)
    for c in range(N_CORES):
        out[c * S:(c + 1) * S] = res.results[c]["y"].reshape(S, H, W, C)
    lg = np.concatenate([res.results[c]["lg"] for c in range(N_CORES)], axis=0)
    return out, lg


def run_classify(x):
    """x: [B, H, W, C] -> logits [B, 4] computed on HW."""
    B = x.shape[0]
    S = B // N_CORES
    # fold the 1/(H*W) mean normalization into W
    wts = np.ascontiguousarray(
        (_W_CLS / float(H * W)).T.reshape(1, 4 * C)).astype(np.float32)
    bias = np.ascontiguousarray(_B_CLS.reshape(1, 4)).astype(np.float32)
    in_maps = []
    for c in range(N_CORES):
        xs = np.ascontiguousarray(x[c * S:(c + 1) * S].reshape(S * H, ROW))
        in_maps.append({"x": xs, "wts": wts, "bias": bias})
    nc = get_classify_nc(S)
    res = run_bass_kernel_spmd(nc, in_maps, core_ids=list(range(N_CORES)))
    lg = np.concatenate([res.results[c]["lg"] for c in range(N_CORES)], axis=0)
    return lg, res


def run_rotate(x, pattern):
    B = x.shape[0]
    S = B // N_CORES
    ident = np.eye(128, dtype=np.float32)
    in_maps = []
    for c in range(N_CORES):
        xs = np.ascontiguousarray(x[c * S:(c + 1) * S].reshape(S * H, ROW))
        in_maps.append({"x": xs, "ident": ident})
    nc = get_rotate_nc(S, pattern)
    res = run_bass_kernel_spmd(nc, in_maps, core_ids=list(range(N_CORES)))
    out = np.empty((B, H, W, C), dtype=np.float32)
    for c in range(N_CORES):
        out[c * S:(c + 1) * S] = res.results[c]["y"].reshape(S, H, W, C)
    return out, res


_W_CLS = None
_B_CLS = None


def _np_fallback(x, W_cls, b_cls):
    mean = x.mean(axis=(1, 2))
    ks = np.argmax(mean @ W_cls + b_cls, axis=-1)
    out = np.empty_like(x)
    for i in range(x.shape[0]):
        out[i] = np.rot90(x[i], int(ks[i]), axes=(0, 1))
    return out


def kernel(x: np.ndarray, W_cls: np.ndarray, b_cls: np.ndarray) -> np.ndarray:
    global _W_CLS, _B_CLS
    x = np.asarray(x)
    B = x.shape[0]
    if x.shape != (B, H, W, C) or B % N_CORES != 0:
        return _np_fallback(np.asarray(x, dtype=np.float32),
                            np.asarray(W_cls, dtype=np.float32),
                            np.asarray(b_cls, dtype=np.float32))
    S = B // N_CORES
    x = np.ascontiguousarray(x, dtype=np.float32)
    _W_CLS = np.asarray(W_cls, dtype=np.float32)
    _B_CLS = np.asarray(b_cls, dtype=np.float32)

    # Speculate the routing a priori: the classifier bias dominates the
    # near-zero image means, so argmax(b) predicts k for ~all samples.
    # The single launch rotates per the speculated pattern AND emits the
    # true logits; mispredicted samples are fixed up afterward.
    k_prior = int(np.argmax(_B_CLS))
    pattern = (k_prior,) * S
    out, lg = run_rotate_cls(x, pattern)
    ks = np.argmax(lg, axis=-1).astype(np.int64)       # [B]

    bad = np.flatnonzero(ks != k_prior)
    if bad.size:
        # host fixup for mispredicted samples (rare: means would have to
        # overcome the bias gaps)
        for b in bad:
            out[b] = np.rot90(x[b], int(ks[b]), axes=(0, 1))
    return out


# revision 35
# speedup vs baseline: 1.0132x; 1.0132x over previous
"""Trainium2 Bass kernel for DiscreteRotation (moe_routing).

Per sample: k = argmax(mean_hw(x) @ W + b); out = rot90(x, k, axes=(H,W)).

Pure data parallel over 8 NeuronCores (8 samples each). Runtime branching is
not supported by this execution path, so the routing is SPECULATED: the
classifier bias dominates the near-zero image means of this input regime, so
k_prior = argmax(b) predicts every sample's route. A single static HW launch
rotates every sample per the speculated pattern AND computes the true logits
from the same SBUF-resident data (per-channel pixel sums on DVE, ones-matmul
on PE); the host then argmaxes the logits and fixes up any mispredicted
sample with numpy rot90 (correct for arbitrary inputs, never triggered by
the bias-dominated regime).

Per-slot rotation code (all fully static):
  r=0: load -> store.
  r=1: load -> pixel-reverse within rows (DVE/ACT, negative-stride input
       APs) -> pixel-transpose on PE (per-channel [<=128 x <=128] tiles via
       identity matmul, PSUM -> DVE/ACT copy) -> store.
  r=2: rot90 applied twice.
  r=3: pixel-transpose, copy-out at reversed pixel positions, each output
       half stored as soon as its copies land -> store.
A separate two-launch path (classify program + pattern-compiled rotate
program) is kept for reference/testing.

Every DMA descriptor is a contiguous 28KB image row on both HBM and SBUF
sides, so all transfers run at full HBM rate. SBUF row layout: image row r
lives at partition r%128, slot r//128 (slot0: rows 0..127, slot1: rows
128..223).
"""
import numpy as np
from contextlib import ExitStack

import concourse.bass as bass
import concourse.bacc as bacc
import concourse.tile as tile
import concourse.mybir as mybir
from concourse.bass_utils import run_bass_kernel_spmd

F32 = mybir.dt.float32

N_CORES = 8
H = 224
W = 224
C = 32
ROW = W * C          # 7168 f32 per image row (28672 B)
P0, P1 = 128, 96     # rows in slot 0 / slot 1


def _flip(ap: bass.AP, dim: int) -> bass.AP:
    """Reverse iteration order of one AP dim."""
    pairs = [list(p) for p in ap.ap]
    stride, num = pairs[dim]
    off = ap.offset + stride * (num - 1)
    pairs[dim] = [-stride, num]
    return bass.AP(ap.tensor, off, pairs)


def _bcast_mid(ap2d: bass.AP, n: int) -> bass.AP:
    """[p, f] -> [p, n, f] with stride-0 middle dim."""
    pairs = [list(p) for p in ap2d.ap]
    assert len(pairs) == 2
    return bass.AP(ap2d.tensor, ap2d.offset, [pairs[0], [0, n], pairs[1]])


def _pixview(ap2d: bass.AP) -> bass.AP:
    """[p, ROW-range] -> [p, c(32), j(224)] channel-major pixel view."""
    return ap2d.rearrange("p (j c) -> p c j", j=W, c=C)


# load chunks: (row0, npart, part0, slot)
CHUNKS = [(0, 64, 0, 0), (64, 64, 64, 0), (128, 64, 0, 1), (192, 32, 64, 1)]


def _build_classify(S: int) -> bacc.Bacc:
    nc = bacc.Bacc("TRN2", target_bir_lowering=False, debug=False,
                   num_devices=N_CORES)
    x = nc.dram_tensor("x", [S * H, ROW], F32, kind="ExternalInput").ap()
    wts = nc.dram_tensor("wts", [1, 4 * C], F32, kind="ExternalInput").ap()
    bias = nc.dram_tensor("bias", [1, 4], F32, kind="ExternalInput").ap()
    lg = nc.dram_tensor("lg", [S, 4], F32, kind="ExternalOutput").ap()

    with tile.TileContext(nc) as tc:
        with ExitStack() as ctx:
            cpool = ctx.enter_context(tc.tile_pool(name="consts", bufs=1))
            apool = ctx.enter_context(tc.tile_pool(name="A", bufs=3))
            spool = ctx.enter_context(tc.tile_pool(name="small", bufs=2))
            mpool = ctx.enter_context(
                tc.tile_pool(name="pmean", bufs=2, space="PSUM"))

            wt = cpool.tile([1, 4 * C], F32)
            bt = cpool.tile([1, 4], F32)
            ones = cpool.tile([128, 1], F32)
            nc.sync.dma_start(out=wt[:], in_=wts)
            nc.sync.dma_start(out=bt[:], in_=bias)
            nc.vector.memset(ones[:], 1.0)

            for s in range(S):
                A = apool.tile([128, 2 * ROW], F32, name=f"A{s}", tag="A")
                sums = spool.tile([128, 2 * C], F32, name=f"sm{s}", tag="sm")
                for r0, n, p0, sl in CHUNKS:
                    nc.sync.dma_start(
                        out=A[p0:p0 + n, sl * ROW:sl * ROW + ROW],
                        in_=x[s * H + r0:s * H + r0 + n, :])
                # per slot: contiguous halving adds (full 128-lane width,
                # DVE 2x mode) down to 14 pixels, then a small strided
                # per-channel reduce
                for sl, n in ((0, P0), (1, P1)):
                    base = sl * ROW
                    npix = W
                    while npix % 2 == 0 and npix > 14:
                        half = npix // 2 * C
                        nc.vector.tensor_add(
                            out=A[0:n, base:base + half],
                            in0=A[0:n, base:base + half],
                            in1=A[0:n, base + half:base + 2 * half])
                        npix //= 2
                    av = A[0:n, base:base + npix * C]
                    nc.vector.tensor_reduce(
                        out=sums[0:n, sl * C:(sl + 1) * C],
                        in_=bass.AP(av.tensor, av.offset,
                                    [list(av.ap[0]), [1, C], [C, npix]]),
                        axis=mybir.AxisListType.X, op=mybir.AluOpType.add)
                pm = mpool.tile([1, C], F32, name=f"pm{s}", tag="pm")
                nc.tensor.matmul(pm[0:1, 0:C], lhsT=ones[0:128, 0:1],
                                 rhs=sums[0:128, 0:C], start=True, stop=False)
                nc.tensor.matmul(pm[0:1, 0:C], lhsT=ones[0:96, 0:1],
                                 rhs=sums[0:96, C:2 * C], start=False,
                                 stop=True)
                tmp = spool.tile([1, 4 * C], F32, name=f"tp{s}", tag="tp")
                wv = wt[:].rearrange("p (k c) -> p k c", k=4, c=C)
                nc.vector.tensor_mul(
                    out=tmp[:].rearrange("p (k c) -> p k c", k=4, c=C),
                    in0=_bcast_mid(pm[0:1, 0:C], 4), in1=wv)
                t4 = spool.tile([1, 4], F32, name=f"t4{s}", tag="t4")
                nc.vector.tensor_reduce(
                    out=t4[:],
                    in_=tmp[:].rearrange("p (k c) -> p k c", k=4, c=C),
                    axis=mybir.AxisListType.X, op=mybir.AluOpType.add)
                l4 = spool.tile([1, 4], F32, name=f"l4{s}", tag="l4")
                nc.vector.tensor_add(out=l4[:], in0=t4[:], in1=bt[:])
                nc.sync.dma_start(out=lg[s:s + 1, :], in_=l4[:])
    nc.finalize()
    return nc


def _build_rotate(S: int, pattern: tuple, with_cls: bool = False) -> bacc.Bacc:
    """Static program: sample slot s gets rotation pattern[s].

    with_cls: additionally compute per-sample logits from the loaded data
    (per-channel pixel sums on DVE, ones-matmul on PE, logit chain) and
    write them to "lg" — lets the host verify a speculated pattern without
    a separate classification pass over x.
    """
    nc = bacc.Bacc("TRN2", target_bir_lowering=False, debug=False,
                   num_devices=N_CORES)
    x = nc.dram_tensor("x", [S * H, ROW], F32, kind="ExternalInput").ap()
    ident = nc.dram_tensor("ident", [128, 128], F32, kind="ExternalInput").ap()
    y = nc.dram_tensor("y", [S * H, ROW], F32, kind="ExternalOutput").ap()
    if with_cls:
        wts = nc.dram_tensor("wts", [1, 4 * C], F32, kind="ExternalInput").ap()
        bias = nc.dram_tensor("bias", [1, 4], F32, kind="ExternalInput").ap()
        lg = nc.dram_tensor("lg", [S, 4], F32, kind="ExternalOutput").ap()

    with tile.TileContext(nc) as tc:
        with ExitStack() as ctx:
            cpool = ctx.enter_context(tc.tile_pool(name="consts", bufs=1))
            apool = ctx.enter_context(tc.tile_pool(name="A", bufs=2))
            bpool = ctx.enter_context(tc.tile_pool(name="B", bufs=1))
            tpool = ctx.enter_context(
                tc.tile_pool(name="ptrans", bufs=6, space="PSUM"))

            it = cpool.tile([128, 128], F32)
            nc.sync.dma_start(out=it[:], in_=ident)
            if with_cls:
                spool = ctx.enter_context(tc.tile_pool(name="small", bufs=2))
                mpool = ctx.enter_context(
                    tc.tile_pool(name="pmean", bufs=2, space="PSUM"))
                wt = cpool.tile([1, 4 * C], F32)
                bt = cpool.tile([1, 4], F32)
                ones = cpool.tile([128, 1], F32)
                nc.sync.dma_start(out=wt[:], in_=wts)
                nc.sync.dma_start(out=bt[:], in_=bias)
                nc.vector.memset(ones[:], 1.0)

            # per-sample load/reduce quarters: (slot, window, nrows, npix)
            QUARTS = [(0, 0, P0, P0), (1, 0, P1, P0),
                      (0, 1, P0, P1), (1, 1, P1, P1)]

            def classify(s, A):
                """Per-channel pixel sums per loaded quarter (full-lane
                strided reduces; each starts as soon as its quarter lands),
                accumulated across quarters by PE -> logits row s."""
                sums = spool.tile([128, 4 * C], F32, name=f"sm{s}", tag="sm")
                for q, (sl, w_, n, npx) in enumerate(QUARTS):
                    off = sl * ROW + w_ * 128 * C
                    av = A[0:n, off:off + npx * C]
                    nc.vector.tensor_reduce(
                        out=sums[0:n, q * C:(q + 1) * C],
                        in_=av.rearrange("p (j c) -> p c j", j=npx, c=C),
                        axis=mybir.AxisListType.X, op=mybir.AluOpType.add)
                pm = mpool.tile([1, C], F32, name=f"pm{s}", tag="pm")
                for q, (sl, w_, n, npx) in enumerate(QUARTS):
                    nc.tensor.matmul(pm[0:1, 0:C], lhsT=ones[0:n, 0:1],
                                     rhs=sums[0:n, q * C:(q + 1) * C],
                                     start=(q == 0), stop=(q == 3))
                tmp = spool.tile([1, 4 * C], F32, name=f"tp{s}", tag="tp")
                wv = wt[:].rearrange("p (k c) -> p k c", k=4, c=C)
                nc.vector.tensor_mul(
                    out=tmp[:].rearrange("p (k c) -> p k c", k=4, c=C),
                    in0=_bcast_mid(pm[0:1, 0:C], 4), in1=wv)
                t4 = spool.tile([1, 4], F32, name=f"t4{s}", tag="t4")
                nc.vector.tensor_reduce(
                    out=t4[:],
                    in_=tmp[:].rearrange("p (k c) -> p k c", k=4, c=C),
                    axis=mybir.AxisListType.X, op=mybir.AluOpType.add)
                l4 = spool.tile([1, 4], F32, name=f"l4{s}", tag="l4")
                nc.vector.tensor_add(out=l4[:], in0=t4[:], in1=bt[:])
                nc.sync.dma_start(out=lg[s:s + 1, :], in_=l4[:])

            def load(s, A):
                # quarter loads in window-major order so the w=0 transposes
                # (and their reduces) start as soon as their data lands
                for sl, w_, n, npx in QUARTS:
                    off = sl * ROW + w_ * 128 * C
                    xr = x[s * H + sl * 128:s * H + sl * 128 + n,
                           w_ * 128 * C:w_ * 128 * C + npx * C]
                    nc.sync.dma_start(out=A[0:n, off:off + npx * C], in_=xr)

            def store_fwd(s, src):
                nc.sync.dma_start(out=y[s * H:s * H + P0, :],
                                  in_=src[0:P0, 0:ROW])
                nc.sync.dma_start(out=y[s * H + P0:s * H + H, :],
                                  in_=src[0:P1, ROW:2 * ROW])

            def rev_pixels(src, dst):
                for sl, n, eng in ((0, P0, "dve"), (1, P1, "act")):
                    sv = src[0:n, sl * ROW:sl * ROW + ROW].rearrange(
                        "p (j c) -> p j c", j=W, c=C)
                    dv = dst[0:n, sl * ROW:sl * ROW + ROW].rearrange(
                        "p (j c) -> p j c", j=W, c=C)
                    if eng == "dve":
                        nc.vector.tensor_copy(out=dv, in_=_flip(sv, 1))
                    else:
                        nc.scalar.copy(out=dv, in_=_flip(sv, 1))

            def transpose_pass(s, src, dst, mode, store=None):
                # w outer so each output half completes (and can store)
                # while the other half is still being transposed
                for w, fw in ((0, P0), (1, P1)):  # dst row window
                    for sl, ps in ((0, P0), (1, P1)):   # source row slot
                        for g in range(8):            # channel groups of 4
                            pt = tpool.tile([128, 512], F32,
                                            name=f"pt{s}{g}{sl}{w}", tag="pt")
                            sv = _pixview(src[0:ps, sl * ROW:sl * ROW + ROW])
                            for cc in range(4):
                                ch = g * 4 + cc
                                nc.tensor.transpose(
                                    pt[0:fw, cc * ps:(cc + 1) * ps],
                                    sv[0:ps, ch:ch + 1, w * 128:w * 128 + fw],
                                    it[0:ps, 0:ps])
                            dv = _pixview(dst[0:fw, w * ROW:w * ROW + ROW])
                            if mode == "T":
                                d3 = dv[0:fw, g * 4:(g + 1) * 4,
                                        sl * 128:sl * 128 + ps]
                            else:  # "k3": reversed pixel positions
                                j0 = 96 if sl == 0 else 0
                                d3 = _flip(
                                    dv[0:fw, g * 4:(g + 1) * 4, j0:j0 + ps], 2)
                            src3 = pt[0:fw, 0:4 * ps].rearrange(
                                "p (c j) -> p c j", c=4, j=ps)
                            if g % 2 == 0:
                                nc.vector.tensor_copy(out=d3, in_=src3)
                            else:
                                nc.scalar.copy(out=d3, in_=src3)
                    if store is not None:
                        store(w)

            for s in range(S):
                A = apool.tile([128, 2 * ROW], F32, name=f"A{s}", tag="A")
                load(s, A)
                if with_cls:
                    classify(s, A)
                r = pattern[s]
                if r == 0:
                    store_fwd(s, A)
                    continue
                B = bpool.tile([128, 2 * ROW], F32, name=f"B{s}", tag="B")
                if r == 1:
                    rev_pixels(A, B)
                    transpose_pass(s, B, A, "T")
                    store_fwd(s, A)
                elif r == 2:
                    rev_pixels(A, B)
                    transpose_pass(s, B, A, "T")
                    rev_pixels(A, B)
                    transpose_pass(s, B, A, "T")
                    store_fwd(s, A)
                else:  # r == 3
                    def store_w(w, s=s, B=B):
                        if w == 0:
                            nc.sync.dma_start(out=y[s * H:s * H + P0, :],
                                              in_=B[0:P0, 0:ROW])
                        else:
                            nc.sync.dma_start(
                                out=y[s * H + P0:s * H + H, :],
                                in_=B[0:P1, ROW:2 * ROW])
                    transpose_pass(s, A, B, "k3", store=store_w)
    nc.finalize()
    return nc


_NC_CACHE = {}


def get_classify_nc(S):
    key = ("cls", S)
    if key not in _NC_CACHE:
        _NC_CACHE[key] = _build_classify(S)
    return _NC_CACHE[key]


def get_rotate_nc(S, pattern):
    key = ("rot", S, pattern)
    if key not in _NC_CACHE:
        _NC_CACHE[key] = _build_rotate(S, pattern)
    return _NC_CACHE[key]


def get_rotate_cls_nc(S, pattern):
    key = ("rotcls", S, pattern)
    if key not in _NC_CACHE:
        _NC_CACHE[key] = _build_rotate(S, pattern, with_cls=True)
    return _NC_CACHE[key]


def run_rotate_cls(x, pattern):
    """One launch: rotate per the speculated pattern AND emit logits."""
    B = x.shape[0]
    S = B // N_CORES
    ident = np.eye(128, dtype=np.float32)
    wts = np.ascontiguousarray(
        (_W_CLS / float(H * W)).T.reshape(1, 4 * C)).astype(np.float32)
    bias = np.ascontiguousarray(_B_CLS.reshape(1, 4)).astype(np.float32)
    in_maps = []
    for c in range(N_CORES):
        xs = np.ascontiguousarray(x[c * S:(c + 1) * S].reshape(S * H, ROW))
        in_maps.append({"x": xs, "ident": ident, "wts": wts, "bias": bias})
    nc = get_rotate_cls_nc(S, pattern)
    res = run_bass_kernel_spmd(nc, in_maps, core_ids=list(range(N_CORES)))
    out = np.empty((B, H, W, C), dtype=np.float32)
    for c in range(N_CORES):
        out[c * S:(c + 1) * S] = res.results[c]["y"].reshape(S, H, W, C)
    lg = np.concatenate([res.results[c]["lg"] for c in range(N_CORES)], axis=0)
    return out, lg


def run_classify(x):
    """x: [B, H, W, C] -> logits [B, 4] computed on HW."""
    B = x.shape[0]
    S = B // N_CORES
    # fold the 1/(H*W) mean normalization into W
    wts = np.ascontiguousarray(
        (_W_CLS / float(H * W)).T.reshape(1, 4 * C)).astype(np.float32)
    bias = np.ascontiguousarray(_B_CLS.reshape(1, 4)).astype(np.float32)
    in_maps = []
    for c in range(N_CORES):
        xs = np.ascontiguousarray(x[c * S:(c + 1) * S].reshape(S * H, ROW))
        in_maps.append({"x": xs, "wts": wts, "bias": bias})
    nc = get_classify_nc(S)
    res = run_bass_kernel_spmd(nc, in_maps, core_ids=list(range(N_CORES)))
    lg = np.concatenate([res.results[c]["lg"] for c in range(N_CORES)], axis=0)
    return lg, res


def run_rotate(x, pattern):
    B = x.shape[0]
    S = B // N_CORES
    ident = np.eye(128, dtype=np.float32)
    in_maps = []
    for c in range(N_CORES):
        xs = np.ascontiguousarray(x[c * S:(c + 1) * S].reshape(S * H, ROW))
        in_maps.append({"x": xs, "ident": ident})
    nc = get_rotate_nc(S, pattern)
    res = run_bass_kernel_spmd(nc, in_maps, core_ids=list(range(N_CORES)))
    out = np.empty((B, H, W, C), dtype=np.float32)
    for c in range(N_CORES):
        out[c * S:(c + 1) * S] = res.results[c]["y"].reshape(S, H, W, C)
    return out, res


_W_CLS = None
_B_CLS = None


def _np_fallback(x, W_cls, b_cls):
    mean = x.mean(axis=(1, 2))
    ks = np.argmax(mean @ W_cls + b_cls, axis=-1)
    out = np.empty_like(x)
    for i in range(x.shape[0]):
        out[i] = np.rot90(x[i], int(ks[i]), axes=(0, 1))
    return out


def kernel(x: np.ndarray, W_cls: np.ndarray, b_cls: np.ndarray) -> np.ndarray:
    global _W_CLS, _B_CLS
    x = np.asarray(x)
    B = x.shape[0]
    if x.shape != (B, H, W, C) or B % N_CORES != 0:
        return _np_fallback(np.asarray(x, dtype=np.float32),
                            np.asarray(W_cls, dtype=np.float32),
                            np.asarray(b_cls, dtype=np.float32))
    S = B // N_CORES
    x = np.ascontiguousarray(x, dtype=np.float32)
    _W_CLS = np.asarray(W_cls, dtype=np.float32)
    _B_CLS = np.asarray(b_cls, dtype=np.float32)

    # Speculate the routing a priori: the classifier bias dominates the
    # near-zero image means, so argmax(b) predicts k for ~all samples.
    # The single launch rotates per the speculated pattern AND emits the
    # true logits; mispredicted samples are fixed up afterward.
    k_prior = int(np.argmax(_B_CLS))
    pattern = (k_prior,) * S
    out, lg = run_rotate_cls(x, pattern)
    ks = np.argmax(lg, axis=-1).astype(np.int64)       # [B]

    bad = np.flatnonzero(ks != k_prior)
    if bad.size:
        # host fixup for mispredicted samples (rare: means would have to
        # overcome the bias gaps)
        for b in bad:
            out[b] = np.rot90(x[b], int(ks[b]), axes=(0, 1))
    return out
